# revision 9
# baseline (speedup 1.0000x reference)
"""Region-augmented embedding lookup (MeanEncoder) on 8 TRN2 NeuronCores.

Reference computation (per batch b, position l):
    out[b,l,0,:] = tanh( sum_{j=0..6} W[ seq_pad[b, l+j]*7 + j , :] ) * (seq[b,l]!=0)

Strategy: data parallel. W is replicated on all 8 cores; each core handles
2 of the 16 sequences (4096 positions). Host precomputes the gather row ids
idx[q, j] = window_token*7 + j and the center-token mask; the device does
the heavy part: 7 indirect-DMA gathers (512B rows from the 179MB table in
HBM), a strided add-reduce over the 7 region rows, tanh (with the mask
folded into the activation scale, exact since mask is 0/1), and the store.
"""

import os
import numpy as np

import concourse.bass as bass
import concourse.tile as tile
from concourse import bacc, mybir
from concourse.bass_utils import run_bass_kernel_spmd

VOCAB = 50000
EMB = 128
RADIUS = 3
REGION = 7
B, L, C = 16, 2048, 1
NCORES = 8
SEQ_PER_CORE = B // NCORES           # 2
POS_PER_CORE = SEQ_PER_CORE * L      # 4096
P = 128                              # positions per tile (partition dim)
NTILES = POS_PER_CORE // P           # 32


def _build_nc():
    nc = bacc.Bacc("TRN2", target_bir_lowering=False, debug=False)

    w = nc.declare_dram_parameter("w", [VOCAB * REGION, EMB], mybir.dt.float32, isOutput=False)
    # idx laid out [P, NTILES, REGION] so one contiguous-row DMA loads it all
    idx = nc.declare_dram_parameter("idx", [P, NTILES * REGION], mybir.dt.int32, isOutput=False)
    mask = nc.declare_dram_parameter("mask", [P, NTILES], mybir.dt.float32, isOutput=False)
    out = nc.declare_dram_parameter("out", [NTILES, P, EMB], mybir.dt.float32, isOutput=True)

    from contextlib import ExitStack
    with tile.TileContext(nc) as tc, ExitStack() as ctx:
        const_pool = ctx.enter_context(tc.tile_pool(name="const", bufs=1))
        gpool = ctx.enter_context(tc.tile_pool(name="gather", bufs=6))
        apool = ctx.enter_context(tc.tile_pool(name="acc", bufs=4))
        opool = ctx.enter_context(tc.tile_pool(name="out", bufs=4))

        idx_sb = const_pool.tile([P, NTILES * REGION], mybir.dt.int32)
        mask_sb = const_pool.tile([P, NTILES], mybir.dt.float32)
        nc.sync.dma_start(idx_sb[:], idx.ap())
        nc.sync.dma_start(mask_sb[:], mask.ap())

        for t in range(NTILES):
            g = gpool.tile([P, REGION, EMB], mybir.dt.float32, tag="g")
            # HW indirect DMA semantics: ONE index per dest partition,
            # streaming the dest partition-row length from that base row.
            # So: one gather per region offset j, each [P,1] indices.
            for j in range(REGION):
                nc.gpsimd.indirect_dma_start(
                    out=g[:, j, :],
                    out_offset=None,
                    in_=w.ap(),
                    in_offset=bass.IndirectOffsetOnAxis(
                        ap=idx_sb[:, t * REGION + j: t * REGION + j + 1],
                        axis=0,
                    ),
                )
            acc = apool.tile([P, EMB], mybir.dt.float32, tag="acc")
            # view g as [P, EMB, REGION] (stride permute) and reduce innermost
            gv = g[:].rearrange("p j e -> p e j")
            nc.vector.tensor_reduce(
                out=acc[:], in_=gv, axis=mybir.AxisListType.X, op=mybir.AluOpType.add
            )
            th = apool.tile([P, EMB], mybir.dt.float32, tag="th")
            nc.scalar.activation(th[:], acc[:], mybir.ActivationFunctionType.Tanh)
            o = opool.tile([P, EMB], mybir.dt.float32, tag="o")
            nc.vector.tensor_tensor(
                out=o[:], in0=th[:],
                in1=mask_sb[:, t: t + 1].to_broadcast([P, EMB]),
                op=mybir.AluOpType.mult,
            )
            nc.sync.dma_start(out.ap()[t], o[:])
    nc.compile()
    return nc


def _host_prep(seq, W):
    """Per-core idx/mask arrays in the DMA-friendly layouts."""
    s = seq.reshape(B, L)                      # C == 1
    pad = np.pad(s, ((0, 0), (RADIUS, RADIUS)))
    # windows[b, l, j] = pad[b, l+j] ; gather row = tok*REGION + j
    windows = np.stack([pad[:, j:j + L] for j in range(REGION)], axis=-1)
    idx_full = (windows * REGION + np.arange(REGION, dtype=np.int32)).astype(np.int32)
    mask_full = (s != 0).astype(np.float32)

    in_maps = []
    for c in range(NCORES):
        idx_c = idx_full[c * SEQ_PER_CORE:(c + 1) * SEQ_PER_CORE].reshape(POS_PER_CORE, REGION)
        mask_c = mask_full[c * SEQ_PER_CORE:(c + 1) * SEQ_PER_CORE].reshape(POS_PER_CORE)
        # [pos, j] -> [P, NTILES, REGION] with pos = t*P + p
        idx_r = idx_c.reshape(NTILES, P, REGION).transpose(1, 0, 2).reshape(P, NTILES * REGION)
        mask_r = mask_c.reshape(NTILES, P).transpose(1, 0)
        in_maps.append({
            "w": np.ascontiguousarray(W),
            "idx": np.ascontiguousarray(idx_r),
            "mask": np.ascontiguousarray(mask_r),
        })
    return in_maps


_NC_CACHE = None


def run(seq, W, trace=False, **spmd_kwargs):
    global _NC_CACHE
    if _NC_CACHE is None:
        _NC_CACHE = _build_nc()
    nc = _NC_CACHE
    in_maps = _host_prep(seq, W)
    res = run_bass_kernel_spmd(
        nc, in_maps, core_ids=list(range(NCORES)), trace=trace, **spmd_kwargs
    )
    outs = [r["out"] for r in res.results]                 # each [NTILES, P, EMB]
    full = np.stack(outs, axis=0).reshape(B, L, EMB)[:, :, None, :]
    return full.astype(np.float32), res


def kernel(seq, W):
    out, _ = run(np.asarray(seq), np.asarray(W))
    return out


# revision 10
# speedup vs baseline: 3.1468x; 3.1468x over previous
"""Region-augmented embedding lookup (MeanEncoder) on 8 TRN2 NeuronCores.

Reference computation (per batch b, position l):
    out[b,l,0,:] = tanh( sum_{j=0..6} W[ seq_pad[b, l+j]*7 + j , :] ) * (seq[b,l]!=0)

Strategy: data parallel, W replicated, each core takes 2 of 16 sequences.

Device kernel per tile (122 output positions from 128 gathered tokens):
  1. ONE indirect DMA gathers, for each of 128 consecutive window
     positions v = q0-3+p, the contiguous 7x128 block
     W[tok(v)*7 : tok(v)*7+7, :] (3584B/partition stream -- the TRN2
     indirect DMA consumes one index per dest partition and streams the
     partition row from that base address).
  2. The shifted region-sum out[i] = sum_j G[i+j, seg_j] runs on the
     tensor engine: 7 fp32 matmuls against identity slices
     (lhsT=ID[:, j:j+122]) accumulated in one PSUM tile. Exact fp32.
  3. tanh+mask in one scalar-engine activation: tanh(psum * mask), exact
     since mask is 0/1.
Out-of-sequence window positions use token id 0 (= the reference's pad),
clamped on the host when building the gather index table.
"""

import numpy as np

import concourse.bass as bass
import concourse.tile as tile
from concourse import bacc, mybir
from concourse.bass_utils import run_bass_kernel_spmd

VOCAB = 50000
EMB = 128
RADIUS = 3
REGION = 7
B, L, C = 16, 2048, 1
NCORES = 8
SEQ_PER_CORE = B // NCORES           # 2
P = 128                              # gathered window positions per tile
TOUT = P - (REGION - 1)              # 122 output positions per tile
TILES_PER_SEQ = -(-L // TOUT)        # 17 (16*122=1952, last tile 96)
NTILES = SEQ_PER_CORE * TILES_PER_SEQ  # 34


def _build_nc():
    nc = bacc.Bacc("TRN2", target_bir_lowering=False, debug=False)

    w = nc.declare_dram_parameter("w", [VOCAB * REGION, EMB], mybir.dt.float32, isOutput=False)
    gidx = nc.declare_dram_parameter("gidx", [P, NTILES], mybir.dt.int32, isOutput=False)
    mask = nc.declare_dram_parameter("mask", [P, NTILES], mybir.dt.float32, isOutput=False)
    ident = nc.declare_dram_parameter("ident", [P, P], mybir.dt.float32, isOutput=False)
    out = nc.declare_dram_parameter("out", [SEQ_PER_CORE * L, EMB], mybir.dt.float32, isOutput=True)

    from contextlib import ExitStack
    with tile.TileContext(nc) as tc, ExitStack() as ctx:
        const_pool = ctx.enter_context(tc.tile_pool(name="const", bufs=1))
        gpool = ctx.enter_context(tc.tile_pool(name="gather", bufs=6))
        ppool = ctx.enter_context(tc.tile_pool(name="psum", bufs=4, space="PSUM"))
        opool = ctx.enter_context(tc.tile_pool(name="out", bufs=4))

        gidx_sb = const_pool.tile([P, NTILES], mybir.dt.int32)
        mask_sb = const_pool.tile([P, NTILES], mybir.dt.float32)
        id_sb = const_pool.tile([P, P], mybir.dt.float32)
        nc.sync.dma_start(gidx_sb[:], gidx.ap())
        nc.sync.dma_start(mask_sb[:], mask.ap())
        nc.sync.dma_start(id_sb[:], ident.ap())

        for t in range(NTILES):
            s, k = divmod(t, TILES_PER_SEQ)
            row0 = s * L + k * TOUT
            nrows = min(TOUT, L - k * TOUT)

            g = gpool.tile([P, REGION * EMB], mybir.dt.float32, tag="g")
            nc.gpsimd.indirect_dma_start(
                out=g[:],
                out_offset=None,
                in_=w.ap(),
                in_offset=bass.IndirectOffsetOnAxis(ap=gidx_sb[:, t: t + 1], axis=0),
            )
            psum = ppool.tile([TOUT, EMB], mybir.dt.float32, tag="ps")
            for j in range(REGION):
                nc.tensor.matmul(
                    out=psum[:],
                    lhsT=id_sb[:, j: j + TOUT],
                    rhs=g[:, j * EMB:(j + 1) * EMB],
                    start=(j == 0),
                    stop=(j == REGION - 1),
                )
            o = opool.tile([TOUT, EMB], mybir.dt.float32, tag="o")
            nc.scalar.activation(
                o[:], psum[:], mybir.ActivationFunctionType.Tanh,
                scale=mask_sb[:TOUT, t: t + 1],
            )
            nc.sync.dma_start(out.ap()[row0: row0 + nrows, :], o[:nrows])
    nc.compile()
    return nc


def _host_prep(seq, W):
    s = seq.reshape(B, L)
    mask_full = (s != 0).astype(np.float32)
    ident = np.eye(P, dtype=np.float32)

    in_maps = []
    for c in range(NCORES):
        gidx_r = np.zeros((P, NTILES), np.int32)
        mask_r = np.zeros((P, NTILES), np.float32)
        for t in range(NTILES):
            sq, k = divmod(t, TILES_PER_SEQ)
            b = c * SEQ_PER_CORE + sq
            q0 = k * TOUT
            # gather window positions v = q0-3+p  (clamp -> pad token 0)
            v = q0 - RADIUS + np.arange(P)
            tok = np.where((v >= 0) & (v < L), s[b, np.clip(v, 0, L - 1)], 0)
            gidx_r[:, t] = tok.astype(np.int32) * REGION
            nrows = min(TOUT, L - q0)
            mask_r[:nrows, t] = mask_full[b, q0: q0 + nrows]
        in_maps.append({
            "w": np.ascontiguousarray(W),
            "gidx": gidx_r,
            "mask": mask_r,
            "ident": ident,
        })
    return in_maps


_NC_CACHE = None


def run(seq, W, trace=False, **spmd_kwargs):
    global _NC_CACHE
    if _NC_CACHE is None:
        _NC_CACHE = _build_nc()
    nc = _NC_CACHE
    in_maps = _host_prep(seq, W)
    res = run_bass_kernel_spmd(
        nc, in_maps, core_ids=list(range(NCORES)), trace=trace, **spmd_kwargs
    )
    outs = [r["out"] for r in res.results]                 # each [2*L, EMB]
    full = np.stack(outs, axis=0).reshape(B, L, EMB)[:, :, None, :]
    return full.astype(np.float32), res


def kernel(seq, W):
    out, _ = run(np.asarray(seq), np.asarray(W))
    return out
